# revision 38
# baseline (speedup 1.0000x reference)
"""Trainium2 Bass kernel for the 2-layer GAT node-classification head.

The reference reads only h2[mask_idx] and x[mask_idx], so the computation
collapses to mask_idx's 2-hop in-neighborhood. On top of the baseline's
sparsity shortcut, this version reorders the layer-1 aggregation:

    sum_e alpha[e,h] * (x[src_e] @ W1_h)  ==  (sum_e alpha[e,h] x[src_e]) @ W1_h

so the big per-edge GEMM disappears: we compute z[h,v,:] (alpha-weighted x
sums, one 768-vector per (head, dst-node)) first, then contract the tiny z
against W1.  W1 streams in bf16 (the dominant DMA, halved).  Per core i the
head order is permuted so head i comes first -- the program is identical
across cores (SPMD), only the data differs.

Sharding over 8 cores: layer-2's 6144-long contraction splits by head; each
core produces a [v1n, 4] partial (2 folded class logits + a_src2 + a_dst2).
The partials are summed with ONE ReduceScatter(add): each core replicates its
partial into all 8 shard slots, so every core's scatter shard is the full
8-way sum.  The tiny layer-2 softmax tail runs redundantly on all cores.
"""

import ml_dtypes
import numpy as np

import concourse.bass as bass
import concourse.mybir as mybir
import concourse.tile as tile
from concourse import bacc
from concourse.bass_utils import run_bass_kernel_spmd

NCORES = 8
P = 128
C = 768          # input feature dim
H1 = 8           # layer-1 heads
OUT = 768        # per-head feature dim
KC = C // P      # 6 chunks of 128 over the 768 contraction
W2F = 4          # folded layer-2 cols: [cls0 cls1 a_src2 a_dst2]
FH = OUT // 2    # W1 piece split point (output-feature halves)

f32 = mybir.dt.float32
bf16 = mybir.dt.bfloat16
ACT = mybir.ActivationFunctionType


# ---------------------------------------------------------------- host graph
def _preprocess(edge_index, mask_idx, n_nodes):
    """2-hop in-neighborhood of mask_idx, edge-major with no padding."""
    ei = np.asarray(edge_index).astype(np.int64)
    m = int(np.asarray(mask_idx))
    src_all = np.concatenate([ei[0], np.arange(n_nodes, dtype=np.int64)])
    dst_all = np.concatenate([ei[1], np.arange(n_nodes, dtype=np.int64)])

    s1_src = src_all[np.nonzero(dst_all == m)[0]].tolist()  # in-edges of m
    v1 = list(dict.fromkeys(s1_src))                        # unique sources
    v1n = len(v1)
    v1_row = {v: r for r, v in enumerate(v1)}
    rm = v1_row[m]                                          # m's row (self-loop)
    mult = np.zeros(v1n, np.float32)
    for s in s1_src:
        mult[v1_row[s]] += 1.0

    # S2: in-edges of each v in V1, edge-major concatenated groups
    e_src, e_dst, m01 = [], [], []
    for g, v in enumerate(v1):
        srcs = src_all[np.nonzero(dst_all == v)[0]]
        for s in srcs:
            e_src.append(int(s))
            e_dst.append(v)
            row = np.zeros(v1n, np.float32)
            row[g] = 1.0
            m01.append(row)
    s2e = len(e_src)
    assert 1 <= s2e <= P, f"S2 edge count {s2e} out of range"
    m01 = np.stack(m01, axis=0)                             # [s2e, v1n]

    meta = dict(v1n=v1n, s2e=s2e, rm=rm)
    host = dict(e_src=np.array(e_src), e_dst=np.array(e_dst),
                m01=m01, mult=mult, m=m)
    return meta, host


def _layout(pieces):
    lay, off = {}, 0
    for name, rows, cols in pieces:
        lay[name] = (rows, off, cols)
        off += cols
    return lay, off


def _cstf_layout(meta):
    v1n, s2e = meta["v1n"], meta["s2e"]
    return _layout([
        ("m01", s2e, v1n),
        ("m01T", v1n, s2e),
        ("m01rep", s2e, H1 * v1n),
        ("xsrcT", P, KC * s2e),        # [c, e] chunked
        ("xdstT", P, KC * s2e),
        ("ws1", P, KC * H1 * 2),       # head-permuted, [W | 0.2*W] per core
        ("wd1", P, KC * H1 * 2),
        ("b1t", P, KC),                # b1 block, chunk-major columns
        ("xmT", P, KC + 1),            # x[m] chunks + e0 col for bias fold
        ("wfb", P, (KC + 1) * 2),      # wf_bot chunks + bias3 row
        ("multv", 1, v1n),
    ])


_DEBUG = False


# ---------------------------------------------------------------- bass build
def _build(meta):
    v1n, s2e, rm = meta["v1n"], meta["s2e"], meta["rm"]
    layf, cwf = _cstf_layout(meta)
    ccw = v1n * W2F                    # payload floats (one shard)

    nc = bacc.Bacc("TRN2", target_bir_lowering=False, debug=False,
                   enable_asserts=True, num_devices=NCORES)

    d_cstf = nc.dram_tensor("cstf", [P, cwf], f32, kind="ExternalInput")
    d_xsrc = nc.dram_tensor("xsrc", [s2e, C], f32, kind="ExternalInput")
    d_cstb = nc.dram_tensor("cstb", [P, KC * W2F + 1], bf16,
                            kind="ExternalInput")
    d_w1a = nc.dram_tensor("w1a", [P, KC * FH], bf16, kind="ExternalInput")
    d_w1b = nc.dram_tensor("w1b", [P, KC * FH], bf16, kind="ExternalInput")
    d_res = nc.dram_tensor("res", [1, 2], f32, kind="ExternalOutput")

    with tile.TileContext(nc) as tc:
        with (
            tc.tile_pool(name="const", bufs=1) as cp,
            tc.tile_pool(name="sbuf", bufs=1) as sb,
            tc.tile_pool(name="ps", bufs=1, space="PSUM") as ps,
            tc.tile_pool(name="dram", bufs=1, space="DRAM") as dr,
        ):
            # ---- input DMAs (small consts first: they gate the z path)
            cstf = cp.tile([P, cwf], f32, tag="cstf")
            nc.sync.dma_start(out=cstf[:], in_=d_cstf[:])
            xsrc = cp.tile([s2e, C], f32, tag="xsrc")
            nc.sync.dma_start(out=xsrc[:], in_=d_xsrc[:])
            cstb = cp.tile([P, KC * W2F + 1], bf16, tag="cstb")
            nc.sync.dma_start(out=cstb[:], in_=d_cstb[:])
            # W1 split by OUTPUT-feature halves so the first half's agg
            # accumulation groups close without waiting for the second DMA.
            # Issued on the Pool engine's SWDGE queue: its descriptor
            # generation runs parallel to the HWDGE pipeline of the const
            # DMAs above, so the W1 transfers start ~1.5us earlier.
            w1A = cp.tile([P, KC * FH], bf16, tag="w1A")
            nc.gpsimd.dma_start(out=w1A[:], in_=d_w1a[:])
            w1B = cp.tile([P, KC * FH], bf16, tag="w1B")
            nc.gpsimd.dma_start(out=w1B[:], in_=d_w1b[:])
            w1Av = w1A[:].rearrange("p (k f) -> p k f", k=KC)
            w1Bv = w1B[:].rearrange("p (k f) -> p k f", k=KC)

            def cvf(name):
                rows, off, cols = layf[name]
                return cstf[0:rows, off:off + cols]

            m01_v = cvf("m01")
            m01T_v = cvf("m01T")
            m01r_v = cvf("m01rep").rearrange("e (h v) -> e h v", h=H1)
            xsrcT_v = cvf("xsrcT").rearrange("p (k e) -> p k e", k=KC)
            xdstT_v = cvf("xdstT").rearrange("p (k e) -> p k e", k=KC)
            ws1_v = cvf("ws1").rearrange("p (k h) -> p k h", k=KC)
            wd1_v = cvf("wd1").rearrange("p (k h) -> p k h", k=KC)
            b1t_v = cvf("b1t")
            xmT_v = cvf("xmT")
            wfb_v = cvf("wfb").rearrange("p (k j) -> p k j", j=2)
            multv_v = cvf("multv")
            w2fb_v = cstb[:, 0:KC * W2F].rearrange("p (k j) -> p k j", k=KC)
            onescol_v = cstb[:, KC * W2F:KC * W2F + 1]

            # ---- preload the Exp activation table off the critical path
            dum = sb.tile([1, 1], f32, tag="dum")
            nc.gpsimd.memset(dum[:], 0.0)
            nc.scalar.activation(out=dum[:], in_=dum[:], func=ACT.Exp)

            # ---- oxm = x[m] @ wf_bot + bias3 (independent of everything)
            oxmp = ps.tile([1, 2], f32, tag="oxm", name="oxmp")
            for k in range(KC + 1):
                nc.tensor.matmul(out=oxmp[:], lhsT=xmT_v[:, k:k + 1],
                                 rhs=wfb_v[:, k, :],
                                 start=(k == 0), stop=(k == KC))

            # ---- layer-1 attention logits: x_src@Ws1 + x_dst@Wd1, per edge
            lg = ps.tile([P, 2 * H1], f32, tag="lg", name="lg")
            for k in range(KC):
                nc.tensor.matmul(out=lg[0:s2e, :], lhsT=xsrcT_v[:, k, :],
                                 rhs=ws1_v[:, k, :],
                                 start=(k == 0), stop=False)
            for k in range(KC):
                nc.tensor.matmul(out=lg[0:s2e, :], lhsT=xdstT_v[:, k, :],
                                 rhs=wd1_v[:, k, :],
                                 start=False, stop=(k == KC - 1))
            # lg holds [logits | 0.2*logits]; exp is monotone, so
            # exp(lrelu(x)) = max(exp(x), exp(0.2 x)): one Act + one DVE op
            e16 = sb.tile([P, 2 * H1], f32, tag="e16")
            nc.scalar.activation(out=e16[0:s2e, :], in_=lg[0:s2e, :],
                                 func=ACT.Exp)
            expl = sb.tile([P, H1], f32, tag="expl")
            nc.vector.tensor_tensor(out=expl[0:s2e, :], in0=e16[0:s2e, 0:H1],
                                    in1=e16[0:s2e, H1:2 * H1],
                                    op=mybir.AluOpType.max)

            # ---- segment softmax via one-hot matmuls (no max shift needed:
            # logits are O(1) by construction). t1 runs parallel to gs/recip.
            t1 = sb.tile([P, H1 * v1n], f32, tag="t1")
            t1_v = t1[0:s2e, :].rearrange("e (h v) -> e h v", h=H1)
            eb = expl[0:s2e, :].rearrange("e (h o) -> e h o", o=1).to_broadcast(
                [s2e, H1, v1n])
            nc.vector.tensor_tensor(out=t1_v, in0=eb, in1=m01r_v,
                                    op=mybir.AluOpType.mult)
            gsp = ps.tile([v1n, H1], f32, tag="gs", name="gsp")
            nc.tensor.matmul(out=gsp[:], lhsT=m01_v, rhs=expl[0:s2e, :],
                             start=True, stop=True)
            rgs = sb.tile([v1n, H1], f32, tag="rgs")
            nc.vector.reciprocal(out=rgs[:], in_=gsp[:])
            rgsep = ps.tile([P, H1], f32, tag="rgse", name="rgsep")
            nc.tensor.matmul(out=rgsep[0:s2e, :], lhsT=m01T_v,
                             rhs=rgs[:], start=True, stop=True)
            At = sb.tile([P, H1 * v1n], f32, tag="At")
            At_v = At[0:s2e, :].rearrange("e (h v) -> e h v", h=H1)
            rb = rgsep[0:s2e, :].rearrange("e (h o) -> e h o",
                                           o=1).to_broadcast([s2e, H1, v1n])
            nc.vector.tensor_tensor(out=At_v, in0=t1_v, in1=rb,
                                    op=mybir.AluOpType.mult)

            # ---- zT[c, (h,v)] = sum_e x_src[e, c] * A[e, (h,v)]
            ztp = ps.tile([P, KC * H1 * v1n], f32, tag="zt", name="ztp")
            ztp_v = ztp[:].rearrange("p (k n) -> p k n", k=KC)
            for k in range(KC):
                nc.tensor.matmul(out=ztp_v[:, k, :],
                                 lhsT=xsrc[:, k * P:(k + 1) * P],
                                 rhs=At[0:s2e, :], start=True, stop=True)
            ztb = sb.tile([P, KC * H1 * v1n], bf16, tag="ztb")
            nc.vector.tensor_copy(out=ztb[:], in_=ztp[:])
            ztb_v = ztb[:].rearrange("p (k n) -> p k n", k=KC)

            # ---- aggT[f, v] = b1[f] + sum_c W1[c, f] * z[head0, v, c]
            # (head0 = this core's head via the per-core head permutation;
            #  the b1 outer-product matmul opens each accumulation group)
            aggp = ps.tile([P, KC * v1n], f32, tag="agg", name="aggp")
            aggp_v = aggp[:].rearrange("p (k v) -> p k v", k=KC)
            for fc in range(KC):
                half = w1Av if fc < KC // 2 else w1Bv
                fo = fc * P if fc < KC // 2 else fc * P - FH
                for k in range(KC):
                    nc.tensor.matmul(out=aggp_v[:, fc, :],
                                     lhsT=half[:, k, fo:fo + P],
                                     rhs=ztb_v[:, k, 0:v1n],
                                     start=(k == 0), stop=(k == KC - 1))

            # ---- helu = elu(agg + b1) = (max(a,0) - 1) + exp(min(a,0))
            a1 = sb.tile([P, KC * v1n], f32, tag="a1")
            b1b = b1t_v.rearrange("p (k o) -> p k o", o=1).to_broadcast(
                [P, KC, v1n])
            nc.vector.tensor_tensor(out=a1[:].rearrange("p (k v) -> p k v",
                                                        k=KC),
                                    in0=aggp_v, in1=b1b,
                                    op=mybir.AluOpType.add)
            mn = sb.tile([P, KC * v1n], f32, tag="mn")
            nc.vector.tensor_scalar_min(out=mn[:], in0=a1[:], scalar1=0.0)
            em = sb.tile([P, KC * v1n], f32, tag="em")
            nc.scalar.activation(out=em[:], in_=mn[:], func=ACT.Exp)
            mx = sb.tile([P, KC * v1n], f32, tag="mx")
            nc.vector.tensor_scalar(out=mx[:], in0=a1[:], scalar1=0.0,
                                    scalar2=-1.0, op0=mybir.AluOpType.max,
                                    op1=mybir.AluOpType.add)
            hel = sb.tile([P, KC * v1n], bf16, tag="hel")
            nc.vector.tensor_tensor(out=hel[:], in0=mx[:], in1=em[:],
                                    op=mybir.AluOpType.add)
            hel_v = hel[:].rearrange("p (k v) -> p k v", k=KC)

            # ---- h2f partial for this head block, straight into [1, (v j)]:
            # rj[f, (v,j)] = hel[f, v] * w2f[f, j]; h2f1 = ones^T @ rj
            rj = sb.tile([P, KC * v1n * W2F], bf16, tag="rj")
            rj_v = rj[:].rearrange("p (k v j) -> p k v j", k=KC, v=v1n)
            hb = hel_v.rearrange("p k (v o) -> p k v o", o=1).to_broadcast(
                [P, KC, v1n, W2F])
            wb = w2fb_v.rearrange("p k (o j) -> p k o j", o=1).to_broadcast(
                [P, KC, v1n, W2F])
            nc.vector.tensor_tensor(out=rj_v, in0=hb, in1=wb,
                                    op=mybir.AluOpType.mult)
            h2f1 = ps.tile([1, ccw], f32, tag="h2f", name="h2f1")
            rj_f = rj[:].rearrange("p (k n) -> p k n", k=KC)
            for fc in range(KC):
                nc.tensor.matmul(out=h2f1[:], lhsT=onescol_v,
                                 rhs=rj_f[:, fc, :],
                                 start=(fc == 0), stop=(fc == KC - 1))
            # replicate the partial into all 8 shard slots: after
            # ReduceScatter(add), every core's scatter shard is the 8-way sum
            stg = sb.tile([1, NCORES * ccw], f32, tag="stg")
            h2b = h2f1[:].rearrange("a (o w) -> a o w", o=1).to_broadcast(
                [1, NCORES, ccw])
            nc.vector.tensor_copy(
                out=stg[:].rearrange("a (r w) -> a r w", r=NCORES), in_=h2b)
            cc_in = dr.tile([1, NCORES * ccw], f32, tag="cc_in", name="cc_in")
            cc_out = dr.tile([1, ccw], f32, tag="cc_out", name="cc_out")
            nc.sync.dma_start(out=cc_in[0:1, :], in_=stg[:])
            nc.gpsimd.collective_compute(
                "ReduceScatter", mybir.AluOpType.add,
                replica_groups=[list(range(NCORES))],
                ins=[cc_in.opt()], outs=[cc_out.opt()])
            hs = sb.tile([1, ccw], f32, tag="hs_sb")
            nc.sync.dma_start(out=hs[:], in_=cc_out[0:1, :])

            # ---- layer-2 softmax over mask's in-edges (tiny, redundant)
            hs_vj = hs[:].rearrange("a (v j) -> a v j", v=v1n)
            lt = sb.tile([1, v1n], f32, tag="lt")
            ad2m = hs[0:1, rm * W2F + 3:rm * W2F + 4].to_broadcast([1, v1n])
            as2_v = hs_vj[:, :, 2:3].rearrange("a v o -> a (v o)")
            nc.vector.tensor_tensor(out=lt[:], in0=as2_v, in1=ad2m,
                                    op=mybir.AluOpType.add)
            lq = sb.tile([1, v1n], f32, tag="lq")
            nc.vector.tensor_scalar_mul(out=lq[:], in0=lt[:], scalar1=0.2)
            lrt = sb.tile([1, v1n], f32, tag="lrt")
            nc.vector.tensor_tensor(out=lrt[:], in0=lt[:], in1=lq[:],
                                    op=mybir.AluOpType.max)
            ex2 = sb.tile([1, v1n], f32, tag="ex2")
            nc.scalar.activation(out=ex2[:], in_=lrt[:], func=ACT.Exp)
            ew = sb.tile([1, v1n], f32, tag="ew")
            nc.vector.tensor_tensor(out=ew[:], in0=ex2[:], in1=multv_v,
                                    op=mybir.AluOpType.mult)
            sm = sb.tile([1, 1], f32, tag="sm")
            nc.vector.reduce_sum(out=sm[:], in_=ew[:],
                                 axis=mybir.AxisListType.X)
            wp = sb.tile([1, 2 * v1n], f32, tag="wp")
            wp_v = wp[:].rearrange("a (j v) -> a j v", j=2)
            cls_jv = hs_vj.rearrange("a v j -> a j v")[:, 0:2, :]
            ewb = ew[:].rearrange("a (o v) -> a o v", o=1).to_broadcast(
                [1, 2, v1n])
            nc.vector.tensor_tensor(out=wp_v, in0=cls_jv, in1=ewb,
                                    op=mybir.AluOpType.mult)
            u = sb.tile([1, 2], f32, tag="u")
            nc.vector.reduce_sum(out=u[:], in_=wp_v,
                                 axis=mybir.AxisListType.X)
            rs = sb.tile([1, 1], f32, tag="rs")
            nc.vector.reciprocal(out=rs[:], in_=sm[:])
            res_sb = sb.tile([1, 2], f32, tag="res_sb")
            nc.vector.tensor_scalar_mul(out=res_sb[:], in0=u[:],
                                        scalar1=rs[:])
            nc.vector.tensor_add(out=res_sb[:], in0=res_sb[:], in1=oxmp[:])
            nc.sync.dma_start(out=d_res[:], in_=res_sb[:])

            if _DEBUG:
                dbg_f32 = [("lr", lr, P, H1), ("expl", expl, P, H1),
                           ("At", At, P, H1 * v1n),
                           ("mx", mx, P, KC * v1n),
                           ("em", em, P, KC * v1n),
                           ("stg", stg, 1, NCORES * ccw),
                           ("hs", hs, 1, ccw),
                           ("lt", lt, 1, v1n), ("ex2", ex2, 1, v1n),
                           ("u", u, 1, 2), ("rgs", rgs, v1n, H1)]
                for nm, t, rr, cc2 in dbg_f32:
                    dd = nc.dram_tensor(f"dbg_{nm}", [rr, cc2], f32,
                                        kind="ExternalOutput")
                    nc.sync.dma_start(out=dd[:], in_=t[0:rr, 0:cc2])
                for nm, t, rr, cc2 in [("ztb", ztb, P, KC * H1 * v1n),
                                       ("hel", hel, P, KC * v1n)]:
                    dd = nc.dram_tensor(f"dbg_{nm}", [rr, cc2], bf16,
                                        kind="ExternalOutput")
                    nc.sync.dma_start(out=dd[:], in_=t[0:rr, 0:cc2])

    nc.compile()
    return nc


_CACHE = {}


def _get_nc(meta):
    key = repr(sorted(meta.items()))
    if key not in _CACHE:
        _CACHE[key] = _build(meta)
    return _CACHE[key]


def _chunkT(a):
    """[768, n] -> [128, KC*n] chunk-major (row f = k*128 + p)."""
    k, n = a.shape
    assert k == C
    return np.ascontiguousarray(
        a.reshape(KC, P, n).transpose(1, 0, 2).reshape(P, KC * n))


def make_in_maps(**inputs):
    x = np.asarray(inputs["x"], np.float32)
    n_nodes = x.shape[0]
    meta, host = _preprocess(inputs["edge_index"], inputs["mask_idx"], n_nodes)
    v1n, s2e = meta["v1n"], meta["s2e"]

    W1 = np.asarray(inputs["W1"], np.float32)
    att_s1 = np.asarray(inputs["att_src1"], np.float32)
    att_d1 = np.asarray(inputs["att_dst1"], np.float32)
    b1 = np.asarray(inputs["b1"], np.float32)
    W2 = np.asarray(inputs["W2"], np.float32)
    att_s2 = np.asarray(inputs["att_src2"], np.float32)
    att_d2 = np.asarray(inputs["att_dst2"], np.float32)
    b2 = np.asarray(inputs["b2"], np.float32)
    fc_w = np.asarray(inputs["fc_w"], np.float32)
    fc_b = np.asarray(inputs["fc_b"], np.float32)
    cls_w = np.asarray(inputs["cls_w"], np.float32)
    cls_b = np.asarray(inputs["cls_b"], np.float32)

    Ws1 = np.einsum("chf,hf->ch", W1.reshape(C, H1, OUT), att_s1)  # [768, 8]
    Wd1 = np.einsum("chf,hf->ch", W1.reshape(C, H1, OUT), att_d1)
    Ws2 = W2 @ att_s2[0]                                           # [6144]
    Wd2 = W2 @ att_d2[0]
    wf = fc_w @ cls_w                                              # [1536, 2]
    wf_top, wf_bot = wf[:OUT], wf[OUT:]
    w2fold = W2 @ wf_top                                           # [6144, 2]
    bias3 = (b2 @ wf_top + fc_b @ cls_w + cls_b).astype(np.float32)

    xsrc = x[host["e_src"]]                                        # [s2e, 768]
    xdst = x[host["e_dst"]]
    xm = x[host["m"]]

    layf, cwf = _cstf_layout(meta)

    def fill(cst, lay, name, arr):
        rows, off, cols = lay[name]
        assert arr.shape == (rows, cols), (name, arr.shape, (rows, cols))
        cst[0:rows, off:off + cols] = arr

    m01rep = np.tile(host["m01"], (1, H1))                         # [s2e, 8*v1n]
    xmT = np.zeros((P, KC + 1), np.float32)
    xmT[:, 0:KC] = xm.reshape(KC, P).T
    xmT[0, KC] = 1.0

    in_maps = []
    for i in range(NCORES):
        perm = [i] + [h for h in range(H1) if h != i]
        w1blk = W1[:, i * OUT:(i + 1) * OUT]
        w2fblk = np.concatenate(
            [w2fold[i * OUT:(i + 1) * OUT, :],
             Ws2[i * OUT:(i + 1) * OUT, None],
             Wd2[i * OUT:(i + 1) * OUT, None]], axis=1)            # [768, 4]
        wfb = np.zeros((P, (KC + 1) * 2), np.float32)
        wfb[:, 0:KC * 2] = _chunkT(np.ascontiguousarray(wf_bot))
        wfb[0, KC * 2:KC * 2 + 2] = bias3

        cstf = np.zeros((P, cwf), np.float32)
        fill(cstf, layf, "m01", host["m01"])
        fill(cstf, layf, "m01T", np.ascontiguousarray(host["m01"].T))
        fill(cstf, layf, "m01rep", m01rep)
        fill(cstf, layf, "xsrcT", _chunkT(np.ascontiguousarray(xsrc.T)))
        fill(cstf, layf, "xdstT", _chunkT(np.ascontiguousarray(xdst.T)))
        ws1p = Ws1[:, perm]
        wd1p = Wd1[:, perm]
        fill(cstf, layf, "ws1",
             _chunkT(np.concatenate([ws1p, 0.2 * ws1p], axis=1)))
        fill(cstf, layf, "wd1",
             _chunkT(np.concatenate([wd1p, 0.2 * wd1p], axis=1)))
        fill(cstf, layf, "b1t",
             b1[i * OUT:(i + 1) * OUT].reshape(KC, P).T)
        fill(cstf, layf, "xmT", xmT)
        fill(cstf, layf, "wfb", wfb)
        fill(cstf, layf, "multv", host["mult"][None, :])

        cstb = np.ones((P, KC * W2F + 1), np.float32)
        cstb[:, 0:KC * W2F] = _chunkT(w2fblk)
        im = {
            "cstf": cstf,
            "xsrc": np.ascontiguousarray(xsrc),
            "cstb": cstb.astype(ml_dtypes.bfloat16),
            "w1a": _chunkT(np.ascontiguousarray(w1blk[:, 0:FH])).astype(
                ml_dtypes.bfloat16),
            "w1b": _chunkT(np.ascontiguousarray(w1blk[:, FH:OUT])).astype(
                ml_dtypes.bfloat16),
        }
        in_maps.append(im)
    return meta, in_maps


def kernel(**inputs):
    meta, in_maps = make_in_maps(**inputs)
    nc = _get_nc(meta)
    res = run_bass_kernel_spmd(nc, in_maps, core_ids=list(range(NCORES)))
    return res.results[0]["res"].astype(np.float32)
